# revision 18
# baseline (speedup 1.0000x reference)
"""Continuous-kernel CNN (CCNN) Trainium2 Bass kernel.

Batch-parallel over 8 NeuronCores (2 sequences per core). Full inputs in,
full output out; sharding + weight packing happens on host here.

Math per layer (cin = 32, nf = 32, K = 5 lags, MLP 1->16->32->16->1024):
  h1 = leaky(dt * W1 + b1); h2 = leaky(h1 @ W2 + b2); h3 = leaky(h2 @ W3 + b3)
  w  = (h3 @ W4 + b4).reshape(cin, nf) * mask
  out[l, o] = sum_{k, c} x[l - lag_k, c] * w[k, l, c, o] (+ skip); leaky applied.

Device mapping: the bilinear sum_{j,c} W4[j,(c,o)] h3[j,n] x[c,n-k] is done by
building y[(j,c), n] = h3m[j, n] * x[c, n-k] at 128 partitions per c-group
(j = p % 16, c = 8 g + p // 16), then 4 PE matmuls K=128 accumulate out[o, n]
in PSUM over lags. The b4 bias term is folded in via a shifted-sum xmsum;
lag-0 of layer 1 (dt == 0 -> constant kernel) is merged into the skip 1x1 conv.
"""
import numpy as np
import ml_dtypes

import concourse.bass as bass
from concourse import bacc
from concourse import mybir
from concourse.tile import TileContext
from concourse.bass_utils import run_bass_kernel_spmd

BS, L = 16, 2048
CIN, NF, K, NL = 32, 32, 5, 2
V = 51            # NUM_TYPES + 1
H1, H2, H3 = 16, 32, 16
NC_N = 8          # cores
BSH = BS // NC_N  # sequences per core
PAD = 8           # left zero-pad columns for shifted views
PL = PAD + L
C = 512           # free-dim chunk (one PSUM bank)
NCH = L // C
BF = mybir.dt.bfloat16
F32 = mybir.dt.float32
F32R = mybir.dt.float32r
LAGSETS = [list(range(1, K + 1)), list(range(1, K))]  # layer0: 1..5, layer1: 1..4
LR = mybir.ActivationFunctionType.Prelu
CP = mybir.ActivationFunctionType.Copy
MUL = mybir.AluOpType.mult
ADD = mybir.AluOpType.add
SUB = mybir.AluOpType.subtract

_cache = {}


def _leaky_np(x):
    return np.where(x >= 0, x, 0.1 * x)




def _build_nc():
    # capture the Tile scheduler's cost-model end time (predicted kernel ns)
    from concourse.bass_interp import CoreSim
    _orig_sim = CoreSim.simulate
    _times = []

    def _patched(self, *a, **k):
        r = _orig_sim(self, *a, **k)
        try:
            _times.append(float(self.time))
        except Exception:
            pass
        return r

    CoreSim.simulate = _patched
    try:
        nc = _build_nc_inner()
    finally:
        CoreSim.simulate = _orig_sim
    _cache["sim_ns"] = max(_times) if _times else None
    return nc


def _build_nc_inner():
    nc = bacc.Bacc(trn_type="TRN2")
    times_d = nc.dram_tensor("times", [BSH, L], F32, kind="ExternalInput")
    types_d = nc.dram_tensor("types", [BSH, L], F32, kind="ExternalInput")
    lens_d = nc.dram_tensor("lens", [1, BSH], F32, kind="ExternalInput")
    emb_d = nc.dram_tensor("embt", [V, CIN], F32, kind="ExternalInput")
    wb1_d = nc.dram_tensor("wb1", [K * H1, 2 * NL], F32, kind="ExternalInput")
    w2b_d = nc.dram_tensor("w2b", [(K - 1) * H1, (K - 1) * H2 * NL], F32R, kind="ExternalInput")
    w25_d = nc.dram_tensor("w25", [H1, H2], F32R, kind="ExternalInput")
    w3b_d = nc.dram_tensor("w3b", [(K - 1) * H2, (K - 1) * H3 * NL], F32R, kind="ExternalInput")
    w35_d = nc.dram_tensor("w35", [H2, H3], F32R, kind="ExternalInput")
    w4p_d = nc.dram_tensor("w4p", [128, NL * 4 * NF], BF, kind="ExternalInput")
    b4r_d = nc.dram_tensor("b4r", [CIN, NL * NF], BF, kind="ExternalInput")
    wsk_d = nc.dram_tensor("wsk", [CIN, NF], BF, kind="ExternalInput")
    bsk_d = nc.dram_tensor("bsk", [NF, 1], F32, kind="ExternalInput")
    rad_d = nc.dram_tensor("rad", [CIN, 4 * 128], BF, kind="ExternalInput")
    b23_d = nc.dram_tensor("b23", [128, 2 * NL], F32, kind="ExternalInput")
    out_d = nc.dram_tensor("out", [BSH, NF, L], F32, kind="ExternalOutput")

    with TileContext(nc) as tc:
        with (
            tc.tile_pool(name="const", bufs=1) as cpool,
            tc.tile_pool(name="seq", bufs=1) as spool,
            tc.tile_pool(name="work", bufs=3) as wpool,
            tc.tile_pool(name="ps1", bufs=1, space="PSUM") as ps1,
            tc.tile_pool(name="ps2", bufs=2, space="PSUM") as ps2,
        ):
            # ---- constants to SBUF (once) ----
            def cload(name, dram, shape, dt):
                t = cpool.tile(shape, dt, tag=name)
                nc.sync.dma_start(t[:], dram[:])
                return t

            embs = cload("embs", emb_d, [V, CIN], F32)
            wb1s = cload("wb1s", wb1_d, [K * H1, 2 * NL], F32)
            w2bs = cload("w2bs", w2b_d, [(K - 1) * H1, (K - 1) * H2 * NL], F32R)
            w25s = cload("w25s", w25_d, [H1, H2], F32R)
            w3bs = cload("w3bs", w3b_d, [(K - 1) * H2, (K - 1) * H3 * NL], F32R)
            w35s = cload("w35s", w35_d, [H2, H3], F32R)
            w4ps = cload("w4ps", w4p_d, [128, NL * 4 * NF], BF)
            b4rs = cload("b4rs", b4r_d, [CIN, NL * NF], BF)
            wsks = cload("wsks", wsk_d, [CIN, NF], BF)
            bsks = cload("bsks", bsk_d, [NF, 1], F32)
            rads = cload("rads", rad_d, [CIN, 4 * 128], BF)
            b23s = cload("b23s", b23_d, [128, 2 * NL], F32)
            lens = cload("lens", lens_d, [1, BSH], F32)

            iotaL = cpool.tile([1, L], mybir.dt.int32, tag="iotaL")
            nc.gpsimd.iota(iotaL[:], pattern=[[1, L]], base=0, channel_multiplier=0)
            iotaLf = cpool.tile([1, L], F32, tag="iotaLf")
            nc.vector.tensor_copy(iotaLf[:], iotaL[:])
            iotaV = cpool.tile([V, 1], mybir.dt.int32, tag="iotaV")
            nc.gpsimd.iota(iotaV[:], pattern=[[1, 1]], base=0, channel_multiplier=1)
            iotaVf = cpool.tile([V, 1], F32, tag="iotaVf")
            nc.vector.tensor_copy(iotaVf[:], iotaV[:])
            al = cpool.tile([128, 1], F32, tag="al")
            nc.vector.memset(al[:], 0.1)

            for b in range(BSH):
                # ---- per-sequence prep ----
                T80 = spool.tile([K * H1, PL], F32, tag="T80", bufs=2)
                nc.vector.memset(T80[0:1, 0:PAD], 0.0)
                nc.sync.dma_start(T80[0:1, PAD:PL], times_d[b : b + 1, :])
                nc.gpsimd.partition_broadcast(T80[:, :], T80[0:1, :])

                # Tsh80 row-block k holds t[l - (k+1)] at column PAD + l
                Tsh80 = spool.tile([K * H1, PL], F32, tag="Tsh80", bufs=2)
                nc.vector.memset(Tsh80[:, :], 0.0)
                dma_engs = [nc.sync, nc.gpsimd, nc.sync, nc.gpsimd, nc.sync]
                for kk in range(1, K + 1):
                    r0 = (kk - 1) * H1
                    src = times_d[b : b + 1, 0 : L - kk].partition_broadcast(H1)
                    dst = Tsh80[r0 : r0 + H1, PAD + kk : PL]
                    dma_engs[kk - 1].dma_start(
                        dst.rearrange("(a p) f -> a p f", a=H1), src,
                    )

                P80 = spool.tile([K * H1, PL], BF, tag="P80", bufs=2)
                nc.vector.memset(P80[0:1, 0:PAD], 0.0)
                nc.vector.tensor_scalar(
                    out=P80[0:1, PAD:PL], in0=iotaLf[:], scalar1=lens[0:1, b : b + 1],
                    scalar2=None, op0=mybir.AluOpType.is_lt,
                )
                nc.gpsimd.partition_broadcast(P80[:, :], P80[0:1, :])

                dt80 = spool.tile([K * H1, L], F32, tag="dt80", bufs=2)
                nc.vector.tensor_tensor(
                    out=dt80[:, :], in0=T80[:, PAD:PL], in1=Tsh80[:, PAD:PL], op=SUB,
                )

                # ---- layer-0 input via one-hot matmul gather (exact fp32) ----
                typr = spool.tile([V, L], F32, tag="typr")
                nc.sync.dma_start(typr[0:1, :], types_d[b : b + 1, :])
                nc.gpsimd.partition_broadcast(typr[:, :], typr[0:1, :])
                oh = spool.tile([V, L], F32, tag="oh")
                nc.vector.tensor_scalar(
                    out=oh[:], in0=typr[:], scalar1=iotaVf[:, 0:1], scalar2=None,
                    op0=mybir.AluOpType.is_equal,
                )
                xs = spool.tile([CIN, PL], BF, tag="xs")
                nc.vector.memset(xs[:, 0:PAD], 0.0)
                for q in range(NCH):
                    gp = ps1.tile([CIN, C], F32, tag="hp3")
                    nc.tensor.matmul(
                        gp[:, :], embs[:, :], oh[:, q * C : (q + 1) * C],
                        start=True, stop=True,
                    )
                    nc.scalar.activation(xs[:, PAD + q * C : PAD + (q + 1) * C], gp[:], CP)

                for li in range(NL):
                    lags = LAGSETS[li]
                    nlag = len(lags)
                    R1 = nlag * H1
                    # ---- h1 for all lags in one ACT op ----
                    h1 = spool.tile([K * H1, L], F32R, tag="h1")
                    nc.scalar.activation(
                        h1[0:R1, :], dt80[0:R1, :], LR,
                        bias=wb1s[0:R1, 2 * li + 1 : 2 * li + 2],
                        scale=wb1s[0:R1, 2 * li : 2 * li + 1], alpha=al[0:R1, 0:1],
                    )
                    # ---- h2, h3: block-diagonal over 4 lags (+ 5th single) ----
                    h3s = spool.tile([K * H1, L], BF, tag="h3s")
                    h3s5 = spool.tile([H3, L], BF, tag="h3s5")
                    for q in range(NCH):
                        c0, c1 = q * C, (q + 1) * C
                        h2p = ps2.tile([128, C], F32, tag="hp2")
                        nc.tensor.matmul(
                            h2p[:, :], (w2bs[:, li * 128 : li * 128 + 128]),
                            (h1[0 : 4 * H1, c0:c1]), start=True, stop=True,
                        )
                        h2s = wpool.tile([128, C], F32R, tag="h2s")
                        nc.scalar.activation(h2s[:], h2p[:], LR, bias=b23s[:, 2 * li : 2 * li + 1], alpha=al[:, 0:1])
                        h3p = ps1.tile([K * H3, C], F32, tag="hp3")
                        nc.tensor.matmul(
                            h3p[0 : 4 * H3, :], (w3bs[:, li * 64 : li * 64 + 64]),
                            (h2s[:, :]), start=True, stop=True,
                        )
                        if nlag == K:
                            h15 = wpool.tile([H1, C], F32R, tag="h15")
                            nc.vector.tensor_copy(h15[:, :], h1[4 * H1 : K * H1, c0:c1])
                            h2p5 = ps2.tile([H2, C], F32, tag="hp2")
                            nc.tensor.matmul(
                                h2p5[:, :], (w25s[:, :]),
                                (h15[:, :]), start=True, stop=True,
                            )
                            h2s5 = wpool.tile([H2, C], F32R, tag="h2s5")
                            nc.scalar.activation(h2s5[:], h2p5[:], LR, bias=b23s[0:H2, 2 * li : 2 * li + 1], alpha=al[0:H2, 0:1])
                            h3p5 = ps1.tile([H3, C], F32, tag="hp25")
                            nc.tensor.matmul(
                                h3p5[:, :], (w35s[:, :]),
                                (h2s5[:, :]), start=True, stop=True,
                            )
                            nc.scalar.activation(
                                h3s5[:, c0:c1], h3p5[:, :],
                                LR, bias=b23s[0:H3, 2 * li + 1 : 2 * li + 2],
                                alpha=al[0:H3, 0:1],
                            )
                        nc.scalar.activation(
                            h3s[0 : 4 * H3, c0:c1], h3p[0 : 4 * H3, :],
                            LR, bias=b23s[0 : 4 * H3, 2 * li + 1 : 2 * li + 2],
                            alpha=al[0 : 4 * H3, 0:1],
                        )

                    h3m = spool.tile([K * H1, L], BF, tag="h3m")
                    nc.vector.tensor_tensor(
                        out=h3m[0 : 4 * H3, :], in0=h3s[0 : 4 * H3, :],
                        in1=P80[0 : 4 * H3, PAD:PL], op=MUL,
                    )
                    if nlag == K:
                        nc.vector.tensor_tensor(
                            out=h3m[4 * H3 : K * H3, :], in0=h3s5[:, :],
                            in1=P80[0:H3, PAD:PL], op=MUL,
                        )

                    # ---- xR: input replicated to (j, c) partition layout ----
                    xRs = []
                    for g in range(4):
                        xr = spool.tile([128, PL], BF, tag=f"xr{g}")
                        nc.vector.memset(xr[:, 0:PAD], 0.0)
                        for q in range(NCH):
                            xrp = ps2.tile([128, C], F32, tag="hp2")
                            nc.tensor.matmul(
                                xrp[:, :], rads[:, g * 128 : (g + 1) * 128],
                                xs[:, PAD + q * C : PAD + (q + 1) * C],
                                start=True, stop=True,
                            )
                            nc.scalar.activation(
                                xr[:, PAD + q * C : PAD + (q + 1) * C], xrp[:], CP,
                            )
                        xRs.append(xr)

                    # ---- xmsum for the b4 bias term ----
                    xmsum = spool.tile([CIN, L], BF, tag="xmsum")
                    nc.vector.tensor_tensor(
                        out=xmsum[:], in0=xs[:, PAD - lags[0] : PL - lags[0]],
                        in1=xs[:, PAD - lags[1] : PL - lags[1]], op=ADD,
                    )
                    for kk in lags[2:]:
                        nc.vector.tensor_tensor(
                            out=xmsum[:], in0=xmsum[:],
                            in1=xs[:, PAD - kk : PL - kk], op=ADD,
                        )
                    nc.vector.tensor_tensor(
                        out=xmsum[:], in0=xmsum[:], in1=P80[0:CIN, PAD:PL], op=MUL,
                    )

                    # ---- lag loop: y build + W4 matmuls into out psum ----
                    outp = ps1.tile([NF, L], F32, tag="outp")
                    first = [True] * NCH
                    for kk in lags:
                        bk = kk - 1
                        hR = wpool.tile([128, L], BF, tag="hR")
                        src16 = h3m[bk * H3 : (bk + 1) * H3, :]
                        nc.sync.dma_start(hR[0:16, :], src16)
                        nc.gpsimd.dma_start(hR[16:32, :], src16)
                        nc.vector.tensor_copy(hR[32:64, :], hR[0:32, :])
                        nc.vector.tensor_copy(hR[64:128, :], hR[0:64, :])
                        for g in range(4):
                            y = wpool.tile([128, L], BF, tag="y")
                            nc.vector.tensor_tensor(
                                out=y[:], in0=hR[:],
                                in1=xRs[g][:, PAD - kk : PL - kk], op=MUL,
                            )
                            for q in range(NCH):
                                nc.tensor.matmul(
                                    outp[:, q * C : (q + 1) * C],
                                    w4ps[:, (li * 4 + g) * NF : (li * 4 + g + 1) * NF],
                                    y[:, q * C : (q + 1) * C],
                                    start=first[q], stop=False,
                                )
                                first[q] = False
                    for q in range(NCH):
                        nc.tensor.matmul(
                            outp[:, q * C : (q + 1) * C],
                            b4rs[:, li * NF : (li + 1) * NF],
                            xmsum[:, q * C : (q + 1) * C],
                            start=False, stop=(li == 0),
                        )
                    if li == 1:
                        for q in range(NCH):
                            nc.tensor.matmul(
                                outp[:, q * C : (q + 1) * C], wsks[:, :],
                                xs[:, PAD + q * C : PAD + (q + 1) * C],
                                start=False, stop=True,
                            )
                    # ---- crossing: leaky(out [+ bskip]) ----
                    if li == 0:
                        xs = spool.tile([CIN, PL], BF, tag="xs2")
                        nc.vector.memset(xs[:, 0:PAD], 0.0)
                        nc.scalar.activation(xs[:, PAD:PL], outp[:], LR, alpha=al[0:CIN, 0:1])
                    else:
                        outF = spool.tile([NF, L], F32, tag="outF")
                        nc.scalar.activation(
                            outF[:], outp[:], LR, bias=bsks[:, 0:1], alpha=al[0:NF, 0:1],
                        )
                        nc.sync.dma_start(out_d[b, :, :], outF[:])
    nc.finalize()
    return nc


def _pack_params(emb, conv_params):
    emb = np.asarray(emb, np.float32)
    ps = [{k: np.asarray(v, np.float32) for k, v in p.items()} for p in conv_params]
    wb1 = np.zeros((K * H1, 2 * NL), np.float32)
    w2b = np.zeros(((K - 1) * H1, (K - 1) * H2 * NL), np.float32)
    w3b = np.zeros(((K - 1) * H2, (K - 1) * H3 * NL), np.float32)
    w4p = np.zeros((128, NL * 4 * NF), np.float32)
    b23 = np.zeros((128, 2 * NL), np.float32)
    b4r = np.zeros((CIN, NL * NF), np.float32)
    for li, p in enumerate(ps):
        nlag = len(LAGSETS[li])
        wb1[: nlag * H1, 2 * li] = np.tile(p["W1"][0], nlag)
        wb1[: nlag * H1, 2 * li + 1] = np.tile(p["b1"], nlag)
        for t in range(K - 1):
            w2b[t * H1 : (t + 1) * H1, li * 128 + t * H2 : li * 128 + (t + 1) * H2] = p["W2"]
            w3b[t * H2 : (t + 1) * H2, li * 64 + t * H3 : li * 64 + (t + 1) * H3] = p["W3"]
        w4 = p["W4"].reshape(H3, CIN, NF)
        for g in range(4):
            for pp in range(128):
                j, c = pp % 16, 8 * g + pp // 16
                w4p[pp, (li * 4 + g) * NF : (li * 4 + g + 1) * NF] = w4[j, c]
        b4r[:, li * NF : (li + 1) * NF] = p["b4"].reshape(CIN, NF)
        b23[:, 2 * li] = np.tile(p["b2"], 4)
        b23[: K * H3, 2 * li + 1] = np.tile(p["b3"], K)
    # merged skip + lag-0 (dt=0) effective 1x1 conv for layer 1
    p1 = ps[1]
    h = _leaky_np(p1["W1"][0] * 0.0 + p1["b1"])
    h = _leaky_np(h @ p1["W2"] + p1["b2"])
    h = _leaky_np(h @ p1["W3"] + p1["b3"])
    w0 = (h @ p1["W4"] + p1["b4"]).reshape(CIN, NF)
    wsk = p1["Wskip"] + w0
    bsk = p1["bskip"].reshape(NF, 1).astype(np.float32)
    rad = np.zeros((CIN, 4 * 128), np.float32)
    for g in range(4):
        for pp in range(128):
            rad[8 * g + pp // 16, g * 128 + pp] = 1.0
    bf = ml_dtypes.bfloat16
    return {
        "embt": emb.copy(), "wb1": wb1,
        "w2b": w2b, "w25": ps[0]["W2"].copy(),
        "w3b": w3b, "w35": ps[0]["W3"].copy(),
        "w4p": w4p.astype(bf), "b4r": b4r.astype(bf),
        "wsk": wsk.astype(bf), "bsk": bsk, "rad": rad.astype(bf), "b23": b23,
    }


def kernel(event_times, event_types, lengths, emb, conv_params):
    if "nc" not in _cache:
        _cache["nc"] = _build_nc()
    nc = _cache["nc"]
    shared = _pack_params(emb, conv_params)
    times = np.asarray(event_times, np.float32)
    types = np.asarray(event_types)
    lens = np.asarray(lengths)
    in_maps = []
    for c in range(NC_N):
        s = slice(c * BSH, (c + 1) * BSH)
        m = dict(shared)
        m["times"] = times[s].copy()
        m["types"] = types[s].astype(np.float32)
        m["lens"] = lens[s].astype(np.float32).reshape(1, BSH)
        in_maps.append(m)
    res = run_bass_kernel_spmd(nc, in_maps, core_ids=list(range(NC_N)))
    out = np.concatenate([r["out"] for r in res.results], axis=0)  # (16, 32, L)
    return np.ascontiguousarray(out.transpose(0, 2, 1)).astype(np.float32)


# revision 22
# speedup vs baseline: 1.0119x; 1.0119x over previous
"""Continuous-kernel CNN (CCNN) Trainium2 Bass kernel.

Batch-parallel over 8 NeuronCores (2 sequences per core). Full inputs in,
full output out; sharding + weight packing happens on host here.

Math per layer (cin = 32, nf = 32, K = 5 lags, MLP 1->16->32->16->1024):
  h1 = leaky(dt * W1 + b1); h2 = leaky(h1 @ W2 + b2); h3 = leaky(h2 @ W3 + b3)
  w  = (h3 @ W4 + b4).reshape(cin, nf) * mask
  out[l, o] = sum_{k, c} x[l - lag_k, c] * w[k, l, c, o] (+ skip); leaky applied.

Device mapping: the bilinear sum_{j,c} W4[j,(c,o)] h3[j,n] x[c,n-k] is done by
building y[(j,c), n] = h3m[j, n] * x[c, n-k] at 128 partitions per c-group
(j = p % 16, c = 8 g + p // 16), then 4 PE matmuls K=128 accumulate out[o, n]
in PSUM over lags. The b4 bias term is folded in via a shifted-sum xmsum;
lag-0 of layer 1 (dt == 0 -> constant kernel) is merged into the skip 1x1 conv.
"""
import numpy as np
import ml_dtypes

import concourse.bass as bass
from concourse import bacc
from concourse import mybir
from concourse.tile import TileContext
from concourse.bass_utils import run_bass_kernel_spmd

BS, L = 16, 2048
CIN, NF, K, NL = 32, 32, 5, 2
V = 51            # NUM_TYPES + 1
H1, H2, H3 = 16, 32, 16
NC_N = 8          # cores
BSH = BS // NC_N  # sequences per core
PAD = 8           # left zero-pad columns for shifted views
PL = PAD + L
C = 512           # free-dim chunk (one PSUM bank)
NCH = L // C
BF = mybir.dt.bfloat16
F32 = mybir.dt.float32
F32R = mybir.dt.float32r
LAGSETS = [list(range(1, K + 1)), list(range(1, K))]  # layer0: 1..5, layer1: 1..4
LR = mybir.ActivationFunctionType.Prelu
CP = mybir.ActivationFunctionType.Copy
MUL = mybir.AluOpType.mult
ADD = mybir.AluOpType.add
SUB = mybir.AluOpType.subtract

_cache = {}


def _leaky_np(x):
    return np.where(x >= 0, x, 0.1 * x)




def _build_nc():
    # capture the Tile scheduler's cost-model end time (predicted kernel ns)
    from concourse.bass_interp import CoreSim
    _orig_sim = CoreSim.simulate
    _times = []

    def _patched(self, *a, **k):
        r = _orig_sim(self, *a, **k)
        try:
            _times.append(float(self.time))
        except Exception:
            pass
        return r

    CoreSim.simulate = _patched
    try:
        nc = _build_nc_inner()
    finally:
        CoreSim.simulate = _orig_sim
    _cache["sim_ns"] = max(_times) if _times else None
    return nc


def _build_nc_inner():
    nc = bacc.Bacc(trn_type="TRN2")
    times_d = nc.dram_tensor("times", [BSH, L], F32, kind="ExternalInput")
    types_d = nc.dram_tensor("types", [BSH, L], F32, kind="ExternalInput")
    lens_d = nc.dram_tensor("lens", [1, BSH], F32, kind="ExternalInput")
    emb_d = nc.dram_tensor("embt", [V, CIN], F32, kind="ExternalInput")
    wb1_d = nc.dram_tensor("wb1", [K * H1, 2 * NL], F32, kind="ExternalInput")
    w2b_d = nc.dram_tensor("w2b", [(K - 1) * H1, (K - 1) * H2 * NL], F32R, kind="ExternalInput")
    w25_d = nc.dram_tensor("w25", [H1, H2], F32R, kind="ExternalInput")
    w3b_d = nc.dram_tensor("w3b", [(K - 1) * H2, (K - 1) * H3 * NL], F32R, kind="ExternalInput")
    w35_d = nc.dram_tensor("w35", [H2, H3], F32R, kind="ExternalInput")
    w4p_d = nc.dram_tensor("w4p", [128, NL * 4 * NF], BF, kind="ExternalInput")
    b4r_d = nc.dram_tensor("b4r", [CIN, NL * NF], BF, kind="ExternalInput")
    wsk_d = nc.dram_tensor("wsk", [CIN, NF], BF, kind="ExternalInput")
    bsk_d = nc.dram_tensor("bsk", [NF, 1], F32, kind="ExternalInput")
    rad_d = nc.dram_tensor("rad", [CIN, 4 * 128], BF, kind="ExternalInput")
    b23_d = nc.dram_tensor("b23", [128, 2 * NL], F32, kind="ExternalInput")
    out_d = nc.dram_tensor("out", [BSH, NF, L], F32, kind="ExternalOutput")

    with TileContext(nc) as tc:
        with (
            tc.tile_pool(name="const", bufs=1) as cpool,
            tc.tile_pool(name="seq", bufs=1) as spool,
            tc.tile_pool(name="work", bufs=3) as wpool,
            tc.tile_pool(name="ps1", bufs=1, space="PSUM") as ps1,
            tc.tile_pool(name="ps2", bufs=2, space="PSUM") as ps2,
        ):
            # ---- constants to SBUF (once) ----
            def cload(name, dram, shape, dt):
                t = cpool.tile(shape, dt, tag=name)
                nc.sync.dma_start(t[:], dram[:])
                return t

            embs = cload("embs", emb_d, [V, CIN], F32)
            wb1s = cload("wb1s", wb1_d, [K * H1, 2 * NL], F32)
            w2bs = cload("w2bs", w2b_d, [(K - 1) * H1, (K - 1) * H2 * NL], F32R)
            w25s = cload("w25s", w25_d, [H1, H2], F32R)
            w3bs = cload("w3bs", w3b_d, [(K - 1) * H2, (K - 1) * H3 * NL], F32R)
            w35s = cload("w35s", w35_d, [H2, H3], F32R)
            w4ps = cload("w4ps", w4p_d, [128, NL * 4 * NF], BF)
            b4rs = cload("b4rs", b4r_d, [CIN, NL * NF], BF)
            wsks = cload("wsks", wsk_d, [CIN, NF], BF)
            bsks = cload("bsks", bsk_d, [NF, 1], F32)
            rads = cload("rads", rad_d, [CIN, 4 * 128], BF)
            b23s = cload("b23s", b23_d, [128, 2 * NL], F32)
            lens = cload("lens", lens_d, [1, BSH], F32)

            iotaL = cpool.tile([1, L], mybir.dt.int32, tag="iotaL")
            nc.gpsimd.iota(iotaL[:], pattern=[[1, L]], base=0, channel_multiplier=0)
            iotaLf = cpool.tile([1, L], F32, tag="iotaLf")
            nc.vector.tensor_copy(iotaLf[:], iotaL[:])
            iotaV = cpool.tile([V, 1], mybir.dt.int32, tag="iotaV")
            nc.gpsimd.iota(iotaV[:], pattern=[[1, 1]], base=0, channel_multiplier=1)
            iotaVf = cpool.tile([V, 1], F32, tag="iotaVf")
            nc.vector.tensor_copy(iotaVf[:], iotaV[:])
            al = cpool.tile([128, 1], F32, tag="al")
            nc.vector.memset(al[:], 0.1)

            for b in range(BSH):
                # ---- per-sequence prep ----
                T80 = spool.tile([K * H1, PL], F32, tag="T80", bufs=2)
                nc.vector.memset(T80[0:1, 0:PAD], 0.0)
                nc.sync.dma_start(T80[0:1, PAD:PL], times_d[b : b + 1, :])
                nc.gpsimd.partition_broadcast(T80[:, :], T80[0:1, :])

                # Tsh80 row-block k holds t[l - (k+1)] at column PAD + l
                Tsh80 = spool.tile([K * H1, PL], F32, tag="Tsh80", bufs=2)
                nc.vector.memset(Tsh80[:, :], 0.0)
                dma_engs = [nc.sync, nc.gpsimd, nc.sync, nc.gpsimd, nc.sync]
                for kk in range(1, K + 1):
                    r0 = (kk - 1) * H1
                    src = times_d[b : b + 1, 0 : L - kk].partition_broadcast(H1)
                    dst = Tsh80[r0 : r0 + H1, PAD + kk : PL]
                    dma_engs[kk - 1].dma_start(
                        dst.rearrange("(a p) f -> a p f", a=H1), src,
                    )

                P80 = spool.tile([K * H1, PL], BF, tag="P80", bufs=2)
                nc.vector.memset(P80[0:1, 0:PAD], 0.0)
                nc.vector.tensor_scalar(
                    out=P80[0:1, PAD:PL], in0=iotaLf[:], scalar1=lens[0:1, b : b + 1],
                    scalar2=None, op0=mybir.AluOpType.is_lt,
                )
                nc.gpsimd.partition_broadcast(P80[:, :], P80[0:1, :])

                dt80 = Tsh80[:, PAD:PL]
                nc.vector.tensor_tensor(
                    out=dt80, in0=T80[:, PAD:PL], in1=Tsh80[:, PAD:PL], op=SUB,
                )

                # ---- layer-0 input via one-hot matmul gather (exact fp32) ----
                oh = spool.tile([V, L], F32, tag="oh")
                nc.sync.dma_start(oh[0:1, :], types_d[b : b + 1, :])
                nc.gpsimd.partition_broadcast(oh[:, :], oh[0:1, :])
                nc.vector.tensor_scalar(
                    out=oh[:], in0=oh[:], scalar1=iotaVf[:, 0:1], scalar2=None,
                    op0=mybir.AluOpType.is_equal,
                )
                xs = spool.tile([CIN, PL], BF, tag="xs")
                nc.vector.memset(xs[:, 0:PAD], 0.0)
                for q in range(NCH):
                    gp = ps1.tile([CIN, C], F32, tag="hp3")
                    nc.tensor.matmul(
                        gp[:, :], embs[:, :], oh[:, q * C : (q + 1) * C],
                        start=True, stop=True,
                    )
                    nc.scalar.activation(xs[:, PAD + q * C : PAD + (q + 1) * C], gp[:], CP)

                for li in range(NL):
                    lags = LAGSETS[li]
                    nlag = len(lags)
                    R1 = nlag * H1
                    # ---- h1 for all lags in one ACT op ----
                    h1 = spool.tile([K * H1, L], F32R, tag="h1", bufs=2)
                    nc.scalar.activation(
                        h1[0:R1, :], dt80[0:R1, :], LR,
                        bias=wb1s[0:R1, 2 * li + 1 : 2 * li + 2],
                        scale=wb1s[0:R1, 2 * li : 2 * li + 1], alpha=al[0:R1, 0:1],
                    )
                    # ---- h2, h3: block-diagonal over 4 lags (+ 5th single) ----
                    h3s = spool.tile([K * H1, L], BF, tag="h3s", bufs=2)
                    h3s5 = spool.tile([H3, L], BF, tag="h3s5")
                    for q in range(NCH):
                        c0, c1 = q * C, (q + 1) * C
                        h2p = ps2.tile([128, C], F32, tag="hp2")
                        nc.tensor.matmul(
                            h2p[:, :], (w2bs[:, li * 128 : li * 128 + 128]),
                            (h1[0 : 4 * H1, c0:c1]), start=True, stop=True,
                        )
                        h2s = wpool.tile([128, C], F32R, tag="h2s")
                        nc.scalar.activation(h2s[:], h2p[:], LR, bias=b23s[:, 2 * li : 2 * li + 1], alpha=al[:, 0:1])
                        h3p = ps1.tile([K * H3, C], F32, tag="hp3")
                        nc.tensor.matmul(
                            h3p[0 : 4 * H3, :], (w3bs[:, li * 64 : li * 64 + 64]),
                            (h2s[:, :]), start=True, stop=True,
                        )
                        if nlag == K:
                            h15 = wpool.tile([H1, C], F32R, tag="h15")
                            nc.vector.tensor_copy(h15[:, :], h1[4 * H1 : K * H1, c0:c1])
                            h2p5 = ps2.tile([H2, C], F32, tag="hp2")
                            nc.tensor.matmul(
                                h2p5[:, :], (w25s[:, :]),
                                (h15[:, :]), start=True, stop=True,
                            )
                            h2s5 = wpool.tile([H2, C], F32R, tag="h2s5")
                            nc.scalar.activation(h2s5[:], h2p5[:], LR, bias=b23s[0:H2, 2 * li : 2 * li + 1], alpha=al[0:H2, 0:1])
                            h3p5 = ps1.tile([H3, C], F32, tag="hp25")
                            nc.tensor.matmul(
                                h3p5[:, :], (w35s[:, :]),
                                (h2s5[:, :]), start=True, stop=True,
                            )
                            nc.scalar.activation(
                                h3s5[:, c0:c1], h3p5[:, :],
                                LR, bias=b23s[0:H3, 2 * li + 1 : 2 * li + 2],
                                alpha=al[0:H3, 0:1],
                            )
                        nc.scalar.activation(
                            h3s[0 : 4 * H3, c0:c1], h3p[0 : 4 * H3, :],
                            LR, bias=b23s[0 : 4 * H3, 2 * li + 1 : 2 * li + 2],
                            alpha=al[0 : 4 * H3, 0:1],
                        )

                    h3m = spool.tile([K * H1, L], BF, tag="h3m", bufs=2)
                    nc.vector.tensor_tensor(
                        out=h3m[0 : 4 * H3, :], in0=h3s[0 : 4 * H3, :],
                        in1=P80[0 : 4 * H3, PAD:PL], op=MUL,
                    )
                    if nlag == K:
                        nc.vector.tensor_tensor(
                            out=h3m[4 * H3 : K * H3, :], in0=h3s5[:, :],
                            in1=P80[0:H3, PAD:PL], op=MUL,
                        )

                    # ---- xR: input replicated to (j, c) partition layout ----
                    xRs = []
                    for g in range(4):
                        xr = spool.tile([128, PL], BF, tag=f"xr{g}")
                        nc.vector.memset(xr[:, 0:PAD], 0.0)
                        for q in range(NCH):
                            xrp = ps2.tile([128, C], F32, tag="hp2")
                            nc.tensor.matmul(
                                xrp[:, :], rads[:, g * 128 : (g + 1) * 128],
                                xs[:, PAD + q * C : PAD + (q + 1) * C],
                                start=True, stop=True,
                            )
                            nc.scalar.activation(
                                xr[:, PAD + q * C : PAD + (q + 1) * C], xrp[:], CP,
                            )
                        xRs.append(xr)

                    # ---- xmsum for the b4 bias term ----
                    xmsum = spool.tile([CIN, L], BF, tag="xmsum")
                    nc.vector.tensor_tensor(
                        out=xmsum[:], in0=xs[:, PAD - lags[0] : PL - lags[0]],
                        in1=xs[:, PAD - lags[1] : PL - lags[1]], op=ADD,
                    )
                    for kk in lags[2:]:
                        nc.vector.tensor_tensor(
                            out=xmsum[:], in0=xmsum[:],
                            in1=xs[:, PAD - kk : PL - kk], op=ADD,
                        )
                    nc.vector.tensor_tensor(
                        out=xmsum[:], in0=xmsum[:], in1=P80[0:CIN, PAD:PL], op=MUL,
                    )

                    # ---- lag loop: y build + W4 matmuls into out psum ----
                    outp = ps1.tile([NF, L], F32, tag="outp")
                    first = [True] * NCH
                    for kk in lags:
                        bk = kk - 1
                        hR = wpool.tile([128, L], BF, tag="hR", bufs=4)
                        src16 = h3m[bk * H3 : (bk + 1) * H3, :]
                        nc.sync.dma_start(hR[0:16, :], src16)
                        nc.gpsimd.dma_start(hR[16:32, :], src16)
                        nc.vector.tensor_copy(hR[32:64, :], hR[0:32, :])
                        nc.vector.tensor_copy(hR[64:128, :], hR[0:64, :])
                        for g in range(4):
                            y = wpool.tile([128, L], BF, tag="y", bufs=4)
                            nc.vector.tensor_tensor(
                                out=y[:], in0=hR[:],
                                in1=xRs[g][:, PAD - kk : PL - kk], op=MUL,
                            )
                            for q in range(NCH):
                                nc.tensor.matmul(
                                    outp[:, q * C : (q + 1) * C],
                                    w4ps[:, (li * 4 + g) * NF : (li * 4 + g + 1) * NF],
                                    y[:, q * C : (q + 1) * C],
                                    start=first[q], stop=False,
                                )
                                first[q] = False
                    for q in range(NCH):
                        nc.tensor.matmul(
                            outp[:, q * C : (q + 1) * C],
                            b4rs[:, li * NF : (li + 1) * NF],
                            xmsum[:, q * C : (q + 1) * C],
                            start=False, stop=(li == 0),
                        )
                    if li == 1:
                        for q in range(NCH):
                            nc.tensor.matmul(
                                outp[:, q * C : (q + 1) * C], wsks[:, :],
                                xs[:, PAD + q * C : PAD + (q + 1) * C],
                                start=False, stop=True,
                            )
                    # ---- crossing: leaky(out [+ bskip]) ----
                    if li == 0:
                        xs = spool.tile([CIN, PL], BF, tag="xs2")
                        nc.vector.memset(xs[:, 0:PAD], 0.0)
                        nc.scalar.activation(xs[:, PAD:PL], outp[:], LR, alpha=al[0:CIN, 0:1])
                    else:
                        outF = spool.tile([NF, L], F32, tag="outF")
                        nc.scalar.activation(
                            outF[:], outp[:], LR, bias=bsks[:, 0:1], alpha=al[0:NF, 0:1],
                        )
                        nc.sync.dma_start(out_d[b, :, :], outF[:])
    nc.finalize()
    return nc


def _pack_params(emb, conv_params):
    emb = np.asarray(emb, np.float32)
    ps = [{k: np.asarray(v, np.float32) for k, v in p.items()} for p in conv_params]
    wb1 = np.zeros((K * H1, 2 * NL), np.float32)
    w2b = np.zeros(((K - 1) * H1, (K - 1) * H2 * NL), np.float32)
    w3b = np.zeros(((K - 1) * H2, (K - 1) * H3 * NL), np.float32)
    w4p = np.zeros((128, NL * 4 * NF), np.float32)
    b23 = np.zeros((128, 2 * NL), np.float32)
    b4r = np.zeros((CIN, NL * NF), np.float32)
    for li, p in enumerate(ps):
        nlag = len(LAGSETS[li])
        wb1[: nlag * H1, 2 * li] = np.tile(p["W1"][0], nlag)
        wb1[: nlag * H1, 2 * li + 1] = np.tile(p["b1"], nlag)
        for t in range(K - 1):
            w2b[t * H1 : (t + 1) * H1, li * 128 + t * H2 : li * 128 + (t + 1) * H2] = p["W2"]
            w3b[t * H2 : (t + 1) * H2, li * 64 + t * H3 : li * 64 + (t + 1) * H3] = p["W3"]
        w4 = p["W4"].reshape(H3, CIN, NF)
        for g in range(4):
            for pp in range(128):
                j, c = pp % 16, 8 * g + pp // 16
                w4p[pp, (li * 4 + g) * NF : (li * 4 + g + 1) * NF] = w4[j, c]
        b4r[:, li * NF : (li + 1) * NF] = p["b4"].reshape(CIN, NF)
        b23[:, 2 * li] = np.tile(p["b2"], 4)
        b23[: K * H3, 2 * li + 1] = np.tile(p["b3"], K)
    # merged skip + lag-0 (dt=0) effective 1x1 conv for layer 1
    p1 = ps[1]
    h = _leaky_np(p1["W1"][0] * 0.0 + p1["b1"])
    h = _leaky_np(h @ p1["W2"] + p1["b2"])
    h = _leaky_np(h @ p1["W3"] + p1["b3"])
    w0 = (h @ p1["W4"] + p1["b4"]).reshape(CIN, NF)
    wsk = p1["Wskip"] + w0
    bsk = p1["bskip"].reshape(NF, 1).astype(np.float32)
    rad = np.zeros((CIN, 4 * 128), np.float32)
    for g in range(4):
        for pp in range(128):
            rad[8 * g + pp // 16, g * 128 + pp] = 1.0
    bf = ml_dtypes.bfloat16
    return {
        "embt": emb.copy(), "wb1": wb1,
        "w2b": w2b, "w25": ps[0]["W2"].copy(),
        "w3b": w3b, "w35": ps[0]["W3"].copy(),
        "w4p": w4p.astype(bf), "b4r": b4r.astype(bf),
        "wsk": wsk.astype(bf), "bsk": bsk, "rad": rad.astype(bf), "b23": b23,
    }


def kernel(event_times, event_types, lengths, emb, conv_params):
    if "nc" not in _cache:
        _cache["nc"] = _build_nc()
    nc = _cache["nc"]
    shared = _pack_params(emb, conv_params)
    times = np.asarray(event_times, np.float32)
    types = np.asarray(event_types)
    lens = np.asarray(lengths)
    in_maps = []
    for c in range(NC_N):
        s = slice(c * BSH, (c + 1) * BSH)
        m = dict(shared)
        m["times"] = times[s].copy()
        m["types"] = types[s].astype(np.float32)
        m["lens"] = lens[s].astype(np.float32).reshape(1, BSH)
        in_maps.append(m)
    res = run_bass_kernel_spmd(nc, in_maps, core_ids=list(range(NC_N)))
    out = np.concatenate([r["out"] for r in res.results], axis=0)  # (16, 32, L)
    return np.ascontiguousarray(out.transpose(0, 2, 1)).astype(np.float32)


# revision 23
# speedup vs baseline: 1.0198x; 1.0078x over previous
"""Continuous-kernel CNN (CCNN) Trainium2 Bass kernel.

Batch-parallel over 8 NeuronCores (2 sequences per core). Full inputs in,
full output out; sharding + weight packing happens on host here.

Math per layer (cin = 32, nf = 32, K = 5 lags, MLP 1->16->32->16->1024):
  h1 = leaky(dt * W1 + b1); h2 = leaky(h1 @ W2 + b2); h3 = leaky(h2 @ W3 + b3)
  w  = (h3 @ W4 + b4).reshape(cin, nf) * mask
  out[l, o] = sum_{k, c} x[l - lag_k, c] * w[k, l, c, o] (+ skip); leaky applied.

Device mapping: the bilinear sum_{j,c} W4[j,(c,o)] h3[j,n] x[c,n-k] is done by
building y[(j,c), n] = h3m[j, n] * x[c, n-k] at 128 partitions per c-group
(j = p % 16, c = 8 g + p // 16), then 4 PE matmuls K=128 accumulate out[o, n]
in PSUM over lags. The b4 bias term is folded in via a shifted-sum xmsum;
lag-0 of layer 1 (dt == 0 -> constant kernel) is merged into the skip 1x1 conv.
"""
import numpy as np
import ml_dtypes

import concourse.bass as bass
from concourse import bacc
from concourse import mybir
from concourse.tile import TileContext
from concourse.bass_utils import run_bass_kernel_spmd

BS, L = 16, 2048
CIN, NF, K, NL = 32, 32, 5, 2
V = 51            # NUM_TYPES + 1
H1, H2, H3 = 16, 32, 16
NC_N = 8          # cores
BSH = BS // NC_N  # sequences per core
PAD = 8           # left zero-pad columns for shifted views
PL = PAD + L
C = 512           # free-dim chunk (one PSUM bank)
NCH = L // C
BF = mybir.dt.bfloat16
F32 = mybir.dt.float32
F32R = mybir.dt.float32r
LAGSETS = [list(range(1, K + 1)), list(range(1, K))]  # layer0: 1..5, layer1: 1..4
LR = mybir.ActivationFunctionType.Prelu
CP = mybir.ActivationFunctionType.Copy
MUL = mybir.AluOpType.mult
ADD = mybir.AluOpType.add
SUB = mybir.AluOpType.subtract

_cache = {}


def _leaky_np(x):
    return np.where(x >= 0, x, 0.1 * x)




def _build_nc():
    # capture the Tile scheduler's cost-model end time (predicted kernel ns)
    from concourse.bass_interp import CoreSim
    _orig_sim = CoreSim.simulate
    _times = []

    def _patched(self, *a, **k):
        r = _orig_sim(self, *a, **k)
        try:
            _times.append(float(self.time))
        except Exception:
            pass
        return r

    CoreSim.simulate = _patched
    try:
        nc = _build_nc_inner()
    finally:
        CoreSim.simulate = _orig_sim
    _cache["sim_ns"] = max(_times) if _times else None
    return nc


def _build_nc_inner():
    nc = bacc.Bacc(trn_type="TRN2")
    times_d = nc.dram_tensor("times", [BSH, L], F32, kind="ExternalInput")
    types_d = nc.dram_tensor("types", [BSH, L], F32, kind="ExternalInput")
    lens_d = nc.dram_tensor("lens", [1, BSH], F32, kind="ExternalInput")
    emb_d = nc.dram_tensor("embt", [V, CIN], F32, kind="ExternalInput")
    wb1_d = nc.dram_tensor("wb1", [K * H1, 2 * NL], F32, kind="ExternalInput")
    w2b_d = nc.dram_tensor("w2b", [(K - 1) * H1, (K - 1) * H2 * NL], F32R, kind="ExternalInput")
    w25_d = nc.dram_tensor("w25", [H1, H2], F32R, kind="ExternalInput")
    w3b_d = nc.dram_tensor("w3b", [(K - 1) * H2, (K - 1) * H3 * NL], F32R, kind="ExternalInput")
    w35_d = nc.dram_tensor("w35", [H2, H3], F32R, kind="ExternalInput")
    w4p_d = nc.dram_tensor("w4p", [128, NL * 4 * NF], BF, kind="ExternalInput")
    b4r_d = nc.dram_tensor("b4r", [CIN, NL * NF], BF, kind="ExternalInput")
    wsk_d = nc.dram_tensor("wsk", [CIN, NF], BF, kind="ExternalInput")
    bsk_d = nc.dram_tensor("bsk", [NF, 1], F32, kind="ExternalInput")
    rad_d = nc.dram_tensor("rad", [CIN, 4 * 128], BF, kind="ExternalInput")
    b23_d = nc.dram_tensor("b23", [128, 2 * NL], F32, kind="ExternalInput")
    out_d = nc.dram_tensor("out", [BSH, NF, L], F32, kind="ExternalOutput")

    with TileContext(nc) as tc:
        with (
            tc.tile_pool(name="const", bufs=1) as cpool,
            tc.tile_pool(name="seq", bufs=1) as spool,
            tc.tile_pool(name="work", bufs=3) as wpool,
            tc.tile_pool(name="ps1", bufs=1, space="PSUM") as ps1,
            tc.tile_pool(name="ps2", bufs=2, space="PSUM") as ps2,
        ):
            # ---- constants to SBUF (once) ----
            def cload(name, dram, shape, dt):
                t = cpool.tile(shape, dt, tag=name)
                nc.sync.dma_start(t[:], dram[:])
                return t

            embs = cload("embs", emb_d, [V, CIN], F32)
            wb1s = cload("wb1s", wb1_d, [K * H1, 2 * NL], F32)
            w2bs = cload("w2bs", w2b_d, [(K - 1) * H1, (K - 1) * H2 * NL], F32R)
            w25s = cload("w25s", w25_d, [H1, H2], F32R)
            w3bs = cload("w3bs", w3b_d, [(K - 1) * H2, (K - 1) * H3 * NL], F32R)
            w35s = cload("w35s", w35_d, [H2, H3], F32R)
            w4ps = cload("w4ps", w4p_d, [128, NL * 4 * NF], BF)
            b4rs = cload("b4rs", b4r_d, [CIN, NL * NF], BF)
            wsks = cload("wsks", wsk_d, [CIN, NF], BF)
            bsks = cload("bsks", bsk_d, [NF, 1], F32)
            rads = cload("rads", rad_d, [CIN, 4 * 128], BF)
            b23s = cload("b23s", b23_d, [128, 2 * NL], F32)
            lens = cload("lens", lens_d, [1, BSH], F32)

            iotaL = cpool.tile([1, L], mybir.dt.int32, tag="iotaL")
            nc.gpsimd.iota(iotaL[:], pattern=[[1, L]], base=0, channel_multiplier=0)
            iotaLf = cpool.tile([1, L], F32, tag="iotaLf")
            nc.vector.tensor_copy(iotaLf[:], iotaL[:])
            iotaV = cpool.tile([V, 1], mybir.dt.int32, tag="iotaV")
            nc.gpsimd.iota(iotaV[:], pattern=[[1, 1]], base=0, channel_multiplier=1)
            iotaVf = cpool.tile([V, 1], F32, tag="iotaVf")
            nc.vector.tensor_copy(iotaVf[:], iotaV[:])
            al = cpool.tile([128, 1], F32, tag="al")
            nc.vector.memset(al[:], 0.1)

            for b in range(BSH):
                # ---- per-sequence prep ----
                T80 = spool.tile([K * H1, PL], F32, tag="T80", bufs=2)
                nc.vector.memset(T80[0:1, 0:PAD], 0.0)
                nc.sync.dma_start(T80[0:1, PAD:PL], times_d[b : b + 1, :])
                nc.gpsimd.partition_broadcast(T80[:, :], T80[0:1, :])

                # Tsh80 row-block k holds t[l - (k+1)] at column PAD + l
                Tsh80 = spool.tile([K * H1, PL], F32, tag="Tsh80", bufs=2)
                nc.vector.memset(Tsh80[:, :], 0.0)
                dma_engs = [nc.sync, nc.gpsimd, nc.sync, nc.gpsimd, nc.sync]
                for kk in range(1, K + 1):
                    r0 = (kk - 1) * H1
                    src = times_d[b : b + 1, 0 : L - kk].partition_broadcast(H1)
                    dst = Tsh80[r0 : r0 + H1, PAD + kk : PL]
                    dma_engs[kk - 1].dma_start(
                        dst.rearrange("(a p) f -> a p f", a=H1), src,
                    )

                P80 = spool.tile([K * H1, PL], BF, tag="P80", bufs=2)
                nc.vector.memset(P80[0:1, 0:PAD], 0.0)
                nc.vector.tensor_scalar(
                    out=P80[0:1, PAD:PL], in0=iotaLf[:], scalar1=lens[0:1, b : b + 1],
                    scalar2=None, op0=mybir.AluOpType.is_lt,
                )
                nc.gpsimd.partition_broadcast(P80[:, :], P80[0:1, :])

                dt80 = Tsh80[:, PAD:PL]
                nc.vector.tensor_tensor(
                    out=dt80, in0=T80[:, PAD:PL], in1=Tsh80[:, PAD:PL], op=SUB,
                )

                # ---- layer-0 input via one-hot matmul gather (exact fp32) ----
                oh = spool.tile([V, L], F32, tag="oh")
                nc.sync.dma_start(oh[0:1, :], types_d[b : b + 1, :])
                nc.gpsimd.partition_broadcast(oh[:, :], oh[0:1, :])
                nc.vector.tensor_scalar(
                    out=oh[:], in0=oh[:], scalar1=iotaVf[:, 0:1], scalar2=None,
                    op0=mybir.AluOpType.is_equal,
                )
                xs = spool.tile([CIN, PL], BF, tag="xs")
                nc.vector.memset(xs[:, 0:PAD], 0.0)
                for q in range(NCH):
                    gp = ps1.tile([CIN, C], F32, tag="hp3")
                    nc.tensor.matmul(
                        gp[:, :], embs[:, :], oh[:, q * C : (q + 1) * C],
                        start=True, stop=True,
                    )
                    nc.scalar.activation(xs[:, PAD + q * C : PAD + (q + 1) * C], gp[:], CP)

                for li in range(NL):
                    lags = LAGSETS[li]
                    nlag = len(lags)
                    R1 = nlag * H1
                    # ---- h1 for all lags in one ACT op ----
                    h1 = spool.tile([K * H1, L], F32R, tag="h1", bufs=2)
                    nc.scalar.activation(
                        h1[0:R1, :], dt80[0:R1, :], LR,
                        bias=wb1s[0:R1, 2 * li + 1 : 2 * li + 2],
                        scale=wb1s[0:R1, 2 * li : 2 * li + 1], alpha=al[0:R1, 0:1],
                    )
                    # ---- h2, h3: block-diagonal over 4 lags (+ 5th single) ----
                    h3s = spool.tile([K * H1, L], BF, tag="h3s", bufs=2)
                    h3s5 = spool.tile([H3, L], BF, tag="h3s5")
                    for q in range(NCH):
                        c0, c1 = q * C, (q + 1) * C
                        h2p = ps2.tile([128, C], F32, tag="hp2")
                        nc.tensor.matmul(
                            h2p[:, :], (w2bs[:, li * 128 : li * 128 + 128]),
                            (h1[0 : 4 * H1, c0:c1]), start=True, stop=True,
                        )
                        h2s = wpool.tile([128, C], F32R, tag="h2s")
                        nc.scalar.activation(h2s[:], h2p[:], LR, bias=b23s[:, 2 * li : 2 * li + 1], alpha=al[:, 0:1])
                        h3p = ps1.tile([K * H3, C], F32, tag="hp3")
                        nc.tensor.matmul(
                            h3p[0 : 4 * H3, :], (w3bs[:, li * 64 : li * 64 + 64]),
                            (h2s[:, :]), start=True, stop=True,
                        )
                        if nlag == K:
                            h15 = wpool.tile([H1, C], F32R, tag="h15")
                            nc.vector.tensor_copy(h15[:, :], h1[4 * H1 : K * H1, c0:c1])
                            h2p5 = ps2.tile([H2, C], F32, tag="hp2")
                            nc.tensor.matmul(
                                h2p5[:, :], (w25s[:, :]),
                                (h15[:, :]), start=True, stop=True,
                            )
                            h2s5 = wpool.tile([H2, C], F32R, tag="h2s5")
                            nc.scalar.activation(h2s5[:], h2p5[:], LR, bias=b23s[0:H2, 2 * li : 2 * li + 1], alpha=al[0:H2, 0:1])
                            h3p5 = ps1.tile([H3, C], F32, tag="hp25")
                            nc.tensor.matmul(
                                h3p5[:, :], (w35s[:, :]),
                                (h2s5[:, :]), start=True, stop=True,
                            )
                            nc.scalar.activation(
                                h3s5[:, c0:c1], h3p5[:, :],
                                LR, bias=b23s[0:H3, 2 * li + 1 : 2 * li + 2],
                                alpha=al[0:H3, 0:1],
                            )
                        nc.scalar.activation(
                            h3s[0 : 4 * H3, c0:c1], h3p[0 : 4 * H3, :],
                            LR, bias=b23s[0 : 4 * H3, 2 * li + 1 : 2 * li + 2],
                            alpha=al[0 : 4 * H3, 0:1],
                        )

                    h3m = spool.tile([K * H1, L], BF, tag="h3m", bufs=2)
                    nc.vector.tensor_tensor(
                        out=h3m[0 : 4 * H3, :], in0=h3s[0 : 4 * H3, :],
                        in1=P80[0 : 4 * H3, PAD:PL], op=MUL,
                    )
                    if nlag == K:
                        nc.vector.tensor_tensor(
                            out=h3m[4 * H3 : K * H3, :], in0=h3s5[:, :],
                            in1=P80[0:H3, PAD:PL], op=MUL,
                        )

                    # ---- xR: input replicated to (j, c) partition layout ----
                    xRs = []
                    for g in range(4):
                        xr = spool.tile([128, PL], BF, tag=f"xr{g}")
                        nc.vector.memset(xr[:, 0:PAD], 0.0)
                        for q in range(NCH):
                            xrp = ps2.tile([128, C], F32, tag="hp2")
                            nc.tensor.matmul(
                                xrp[:, :], rads[:, g * 128 : (g + 1) * 128],
                                xs[:, PAD + q * C : PAD + (q + 1) * C],
                                start=True, stop=True,
                            )
                            nc.scalar.activation(
                                xr[:, PAD + q * C : PAD + (q + 1) * C], xrp[:], CP,
                            )
                        xRs.append(xr)

                    # ---- xmsum for the b4 bias term ----
                    xmsum = spool.tile([CIN, L], BF, tag="xmsum")
                    nc.vector.tensor_tensor(
                        out=xmsum[:], in0=xs[:, PAD - lags[0] : PL - lags[0]],
                        in1=xs[:, PAD - lags[1] : PL - lags[1]], op=ADD,
                    )
                    for kk in lags[2:]:
                        nc.vector.tensor_tensor(
                            out=xmsum[:], in0=xmsum[:],
                            in1=xs[:, PAD - kk : PL - kk], op=ADD,
                        )
                    nc.vector.tensor_tensor(
                        out=xmsum[:], in0=xmsum[:], in1=P80[0:CIN, PAD:PL], op=MUL,
                    )

                    # ---- lag loop: y build + W4 matmuls into out psum ----
                    outp = ps1.tile([NF, L], F32, tag="outp")
                    first = [True] * NCH
                    for kk in lags:
                        bk = kk - 1
                        hR = wpool.tile([128, L], BF, tag="hR", bufs=4)
                        src16 = h3m[bk * H3 : (bk + 1) * H3, :]
                        nc.sync.dma_start(hR[0:16, :], src16)
                        nc.gpsimd.dma_start(hR[16:32, :], src16)
                        nc.vector.tensor_copy(hR[32:64, :], hR[0:32, :])
                        nc.vector.tensor_copy(hR[64:128, :], hR[0:64, :])
                        for g in range(4):
                            y = wpool.tile([128, L], BF, tag="y", bufs=4)
                            nc.vector.tensor_tensor(
                                out=y[:], in0=hR[:],
                                in1=xRs[g][:, PAD - kk : PL - kk], op=MUL,
                            )
                            for q in range(NCH):
                                nc.tensor.matmul(
                                    outp[:, q * C : (q + 1) * C],
                                    w4ps[:, (li * 4 + g) * NF : (li * 4 + g + 1) * NF],
                                    y[:, q * C : (q + 1) * C],
                                    start=first[q], stop=False,
                                )
                                first[q] = False
                    for q in range(NCH):
                        nc.tensor.matmul(
                            outp[:, q * C : (q + 1) * C],
                            b4rs[:, li * NF : (li + 1) * NF],
                            xmsum[:, q * C : (q + 1) * C],
                            start=False, stop=(li == 0),
                        )
                    if li == 1:
                        for q in range(NCH):
                            nc.tensor.matmul(
                                outp[:, q * C : (q + 1) * C], wsks[:, :],
                                xs[:, PAD + q * C : PAD + (q + 1) * C],
                                start=False, stop=True,
                            )
                    # ---- crossing: leaky(out [+ bskip]) ----
                    if li == 0:
                        xs = spool.tile([CIN, PL], BF, tag="xs2")
                        nc.vector.memset(xs[:, 0:PAD], 0.0)
                        nc.scalar.activation(xs[:, PAD:PL], outp[:], LR, alpha=al[0:CIN, 0:1])
                    else:
                        outF = spool.tile([NF, L], F32, tag="outF")
                        nc.scalar.activation(
                            outF[:], outp[:], LR, bias=bsks[:, 0:1], alpha=al[0:NF, 0:1],
                        )
                        nc.sync.dma_start(out_d[b, :, 0 : L // 2], outF[:, 0 : L // 2])
                        nc.gpsimd.dma_start(out_d[b, :, L // 2 : L], outF[:, L // 2 : L])
    nc.finalize()
    return nc


def _pack_params(emb, conv_params):
    emb = np.asarray(emb, np.float32)
    ps = [{k: np.asarray(v, np.float32) for k, v in p.items()} for p in conv_params]
    wb1 = np.zeros((K * H1, 2 * NL), np.float32)
    w2b = np.zeros(((K - 1) * H1, (K - 1) * H2 * NL), np.float32)
    w3b = np.zeros(((K - 1) * H2, (K - 1) * H3 * NL), np.float32)
    w4p = np.zeros((128, NL * 4 * NF), np.float32)
    b23 = np.zeros((128, 2 * NL), np.float32)
    b4r = np.zeros((CIN, NL * NF), np.float32)
    for li, p in enumerate(ps):
        nlag = len(LAGSETS[li])
        wb1[: nlag * H1, 2 * li] = np.tile(p["W1"][0], nlag)
        wb1[: nlag * H1, 2 * li + 1] = np.tile(p["b1"], nlag)
        for t in range(K - 1):
            w2b[t * H1 : (t + 1) * H1, li * 128 + t * H2 : li * 128 + (t + 1) * H2] = p["W2"]
            w3b[t * H2 : (t + 1) * H2, li * 64 + t * H3 : li * 64 + (t + 1) * H3] = p["W3"]
        w4 = p["W4"].reshape(H3, CIN, NF)
        for g in range(4):
            for pp in range(128):
                j, c = pp % 16, 8 * g + pp // 16
                w4p[pp, (li * 4 + g) * NF : (li * 4 + g + 1) * NF] = w4[j, c]
        b4r[:, li * NF : (li + 1) * NF] = p["b4"].reshape(CIN, NF)
        b23[:, 2 * li] = np.tile(p["b2"], 4)
        b23[: K * H3, 2 * li + 1] = np.tile(p["b3"], K)
    # merged skip + lag-0 (dt=0) effective 1x1 conv for layer 1
    p1 = ps[1]
    h = _leaky_np(p1["W1"][0] * 0.0 + p1["b1"])
    h = _leaky_np(h @ p1["W2"] + p1["b2"])
    h = _leaky_np(h @ p1["W3"] + p1["b3"])
    w0 = (h @ p1["W4"] + p1["b4"]).reshape(CIN, NF)
    wsk = p1["Wskip"] + w0
    bsk = p1["bskip"].reshape(NF, 1).astype(np.float32)
    rad = np.zeros((CIN, 4 * 128), np.float32)
    for g in range(4):
        for pp in range(128):
            rad[8 * g + pp // 16, g * 128 + pp] = 1.0
    bf = ml_dtypes.bfloat16
    return {
        "embt": emb.copy(), "wb1": wb1,
        "w2b": w2b, "w25": ps[0]["W2"].copy(),
        "w3b": w3b, "w35": ps[0]["W3"].copy(),
        "w4p": w4p.astype(bf), "b4r": b4r.astype(bf),
        "wsk": wsk.astype(bf), "bsk": bsk, "rad": rad.astype(bf), "b23": b23,
    }


def kernel(event_times, event_types, lengths, emb, conv_params):
    if "nc" not in _cache:
        _cache["nc"] = _build_nc()
    nc = _cache["nc"]
    shared = _pack_params(emb, conv_params)
    times = np.asarray(event_times, np.float32)
    types = np.asarray(event_types)
    lens = np.asarray(lengths)
    in_maps = []
    for c in range(NC_N):
        s = slice(c * BSH, (c + 1) * BSH)
        m = dict(shared)
        m["times"] = times[s].copy()
        m["types"] = types[s].astype(np.float32)
        m["lens"] = lens[s].astype(np.float32).reshape(1, BSH)
        in_maps.append(m)
    res = run_bass_kernel_spmd(nc, in_maps, core_ids=list(range(NC_N)))
    out = np.concatenate([r["out"] for r in res.results], axis=0)  # (16, 32, L)
    return np.ascontiguousarray(out.transpose(0, 2, 1)).astype(np.float32)
